# revision 21
# baseline (speedup 1.0000x reference)
"""Causal self-attention (B=2, T=2048, E=1024, 16 heads) on 8 TRN2 NeuronCores.

Sharding (Megatron-style, zero device-side collectives):
  core c in 0..7 -> batch b = c//4, head group hg = c%4 (4 heads, 256 head-dims).
  Each core computes, for its batch and its 4 heads:
    qT/kT = (w_q|w_k)^T x^T   (transposed layout: [head_dim, T])
    v     = x w_v             (natural layout: [T, head_dim], + ones column)
    sT    = kT^T-block matmuls -> [tk, tq] score blocks (causal blocks only,
            diagonal blocks column-trimmed to the valid causal range)
    causal mask on diagonal squares applied ON THE PE: an extra psum-accumulate
            matmul adds -30000 to the invalid triangle before exp
    expS  = exp(sT/8), fp16, one (possibly strided) activation per key block
    yT    = v_plus^T @ expS -> [65, tq]; row 64 accumulates softmax row-sums
    y_norm= yT[0:64] * broadcast(1/rowsum)  (reciprocal_approx_fast on DVE)
    out_c = y_norm^T w_proj[rows of its heads] -> partial [T, E]; the proj
            psum is DMA'd to DRAM directly in fp32 (no cast pass)
  Host: out[b] = sum of the 4 partials + b_proj + b_v @ w_proj.
  b_k is dropped (softmax is invariant to per-row constants); b_q is applied
  on-device; b_v is folded into the output bias.

Work for chunk c+1's qkv and chunk c-1's projection is interleaved into
chunk c's attention j-loop so the PE queue always holds ready work while
the scalar engine computes exp (keeps the PE busy and its HAM clock warm).
"""

import numpy as np

N_HEAD = 16
E = 1024
B, T = 2, 2048
HD = E // N_HEAD          # 64
N_CORES = 8
HPC = 4                   # heads per core
DJ = HPC * HD             # 256 head-dim columns per core
ET = E // 128             # 8  e-tiles
TT = T // 128             # 16 t-tiles
TC = T // 512             # 4  t-chunks
SCALE = 1.0 / np.sqrt(HD)  # 0.125
MASK_NEG = -30000.0
MASK_MODE = "dve"        # "pe": psum-accumulate matmul; "dve": post-exp multiply
EXP_STRIDED = True       # one strided activation vs two per-half calls

_STATE = {}


def _build_nc(reps=1):
    import concourse.tile as tile
    from concourse import mybir
    from concourse.bacc import Bacc

    f32 = mybir.dt.float32
    f16 = mybir.dt.float16
    AF = mybir.ActivationFunctionType

    nc = Bacc()
    xT_d = nc.dram_tensor("xT", [E, T], f16, kind="ExternalInput")
    wqk_d = nc.dram_tensor("wqk", [E, 2 * DJ], f16, kind="ExternalInput")
    wv_d = nc.dram_tensor("wv", [E, DJ], f16, kind="ExternalInput")
    wp_d = nc.dram_tensor("wp", [DJ, E], f16, kind="ExternalInput")
    bq_d = nc.dram_tensor("bq", [128, 2], f32, kind="ExternalInput")
    mask_d = nc.dram_tensor("mask", [128, 128], f16, kind="ExternalInput")
    ident_d = nc.dram_tensor("ident", [128, 128], f16, kind="ExternalInput")
    ones4_d = nc.dram_tensor("ones4", [128, HPC, 1], f16, kind="ExternalInput")
    out_d = nc.dram_tensor("out", [T, E], f16, kind="ExternalOutput")

    with tile.TileContext(nc) as tc:
        with (
            tc.tile_pool(name="xw", bufs=1) as xw,          # persistent inputs
            tc.tile_pool(name="qkv", bufs=1) as qkv,        # persistent qT/kT/v/yT
            tc.tile_pool(name="es", bufs=4) as esp,         # exp(score) blocks
            tc.tile_pool(name="nrm", bufs=2) as nrm,
            tc.tile_pool(name="ob", bufs=3) as obp,        # norm scratch
            tc.tile_pool(name="mm", bufs=2, space="PSUM") as mmp,   # qkv/v/proj accs
            tc.tile_pool(name="s2", bufs=2, space="PSUM") as s2p,   # score blocks
            tc.tile_pool(name="yps", bufs=1, space="PSUM") as yps,  # PV accumulators
        ):
          for _rep in range(reps):
            # ---- input DMAs: wqk/xT interleaved et-major so the first qk
            # matmuls unblock as early as possible ----
            xT_sb = []
            wqk_sb = []
            wv_sb = []
            for et in range(ET):
                w = xw.tile([128, 2 * DJ], f16, tag=f"wqk{et}", name=f"wqk{et}")
                nc.sync.dma_start(w[:], wqk_d[128 * et : 128 * (et + 1), :])
                wqk_sb.append(w)
                t = xw.tile([128, T], f16, tag=f"xT{et}", name=f"xT{et}")
                nc.scalar.dma_start(
                    t[:, 0:512], xT_d[128 * et : 128 * (et + 1), 0:512]
                )
                xT_sb.append(t)
            # small early tensors first on the SWDGE queue
            bq_sb = xw.tile([128, 2], f32, tag="bq", name="bq")
            nc.gpsimd.dma_start(bq_sb[:], bq_d[:])
            ones4_sb = xw.tile([128, HPC, 1], f16, tag="ones4", name="ones4")
            nc.gpsimd.dma_start(ones4_sb[:], ones4_d[:])
            for et in range(ET):
                t = xw.tile([128, DJ], f16, tag=f"wv{et}", name=f"wv{et}")
                nc.gpsimd.dma_start(t[:], wv_d[128 * et : 128 * (et + 1), :])
                wv_sb.append(t)
            mask_sb = xw.tile([128, 128], f16, tag="mask", name="mask")
            nc.gpsimd.dma_start(mask_sb[:], mask_d[:])
            ident_sb = xw.tile([128, 128], f16, tag="ident", name="ident")
            nc.gpsimd.dma_start(ident_sb[:], ident_d[:])
            wp_sb = []
            for kt in range(2):
                t = xw.tile([128, E], f16, tag=f"wp{kt}", name=f"wp{kt}")
                nc.gpsimd.dma_start(t[:], wp_d[128 * kt : 128 * (kt + 1), :])
                wp_sb.append(t)
            # remaining x chunks on sync (keeps the Act queue free for exp)
            for cq in (1, 2, 3):
                for et in range(ET):
                    nc.sync.dma_start(
                        xT_sb[et][:, 512 * cq : 512 * (cq + 1)],
                        xT_d[128 * et : 128 * (et + 1), 512 * cq : 512 * (cq + 1)],
                    )
            if reps > 1 and _rep > 0:
                # measurement builds: serialize reps by folding a read-back
                # sampling EVERY output t-tile of the previous rep into the
                # v ones-column (the perturbation rounds away in fp16; the
                # data dependency is what serializes)
                chain = xw.tile([128, TT, 4], f16, tag="chain", name="chain")
                nc.sync.dma_start(
                    chain[:],
                    out_d.rearrange("(n p) e -> p n e", p=128)[:, :, 0:4],
                )
                red = xw.tile([128, 1], f32, tag="red", name="red")
                nc.vector.tensor_reduce(
                    out=red[:], in_=chain[:], axis=mybir.AxisListType.XY,
                    op=mybir.AluOpType.add,
                )
                o4b = xw.tile([128, HPC, 1], f16, tag="ones4b", name="ones4b")
                rs = xw.tile([128, 1], f32, tag="rs", name="rs")
                nc.vector.tensor_scalar_mul(rs[:], red[:], 1e-7)
                with nc.allow_low_precision(reason="timing chain"):
                    nc.vector.tensor_scalar_add(o4b[:], ones4_sb[:], rs[:])
                ones4_sb = o4b

            # PE warm-up spin: dummy matmuls on memset scratch (output
            # never read) open the HAM clock-gate to full rate while input
            # DMAs stream; parked in the (idle-at-start) s2 psum pool
            scr = xw.tile([128, 512], f16, tag="scr", name="scr")
            nc.vector.memset(scr[:], 0.0)

            def spin_mm(n=1):
                sp = s2p.tile([128, 1024], f32, tag="s2", name="spin")
                for _ in range(n):
                    nc.tensor.matmul(sp[:, 0:512], scr[:, 0:128], scr[:],
                                     start=True, stop=True)

            spin_mm(12)

            # warm the exp activation table immediately (gated only on the
            # scratch memset, well before the first real exp)
            warm = nrm.tile([1, 1], f16, tag="warm", name="warm")
            nc.scalar.activation(out=warm[:], in_=scr[0:1, 0:1],
                                 func=AF.Exp, scale=1.0)

            # persistent intermediates (fp16)
            qT_sb = [qkv.tile([128, T], f16, tag=f"qT{i}", name=f"qT{i}") for i in range(2)]
            kT_sb = [qkv.tile([128, T], f16, tag=f"kT{i}", name=f"kT{i}") for i in range(2)]
            v_sb = [qkv.tile([128, HPC, HD + 1], f16, tag=f"v{i}", name=f"v{i}") for i in range(TT)]
            yT_sb = [qkv.tile([128, T], f16, tag=f"yT{i}", name=f"yT{i}") for i in range(2)]

            def qk_block(c, jt):
                # qT/kT rows 128*jt..128*jt+128, query/key chunk c
                acc = mmp.tile([128, 512], f32, tag="mm", name="acc_qk")
                for et in range(ET):
                    nc.tensor.matmul(
                        acc[:],
                        wqk_sb[et][:, 128 * jt : 128 * (jt + 1)],
                        xT_sb[et][:, 512 * c : 512 * (c + 1)],
                        start=(et == 0),
                        stop=(et == ET - 1),
                    )
                if jt < 2:
                    # q: add bias while casting out of PSUM (DVE)
                    nc.vector.tensor_scalar_add(
                        qT_sb[jt][:, 512 * c : 512 * (c + 1)],
                        acc[:],
                        bq_sb[:, jt : jt + 1],
                    )
                else:
                    nc.vector.tensor_copy(
                        kT_sb[jt - 2][:, 512 * c : 512 * (c + 1)], acc[:]
                    )

            def v_block(c, tt):
                acc = mmp.tile([128, 512], f32, tag="mm", name="acc_v")
                for et in range(ET):
                    nc.tensor.matmul(
                        acc[:, 0:DJ],
                        xT_sb[et][:, 128 * tt : 128 * (tt + 1)],
                        wv_sb[et][:],
                        start=(et == 0),
                        stop=(et == ET - 1),
                    )
                nc.vector.tensor_copy(
                    v_sb[tt][:, :, 0:HD],
                    acc[:, 0:DJ].rearrange("p (h d) -> p h d", h=HPC),
                )
                nc.gpsimd.tensor_copy(v_sb[tt][:, :, HD : HD + 1], ones4_sb[:])

            def proj_block(c, tt, nk):
                # out[128t, 512e] = yT[:, t-tile]^T @ wp[:, nk-half];
                # cast alternates DVE / Act-Copy to balance engine load
                acc = mmp.tile([128, 512], f32, tag="mm", name="acc_p")
                for kt in range(2):
                    nc.tensor.matmul(
                        acc[:],
                        yT_sb[kt][:, 128 * tt : 128 * (tt + 1)],
                        wp_sb[kt][:, 512 * nk : 512 * (nk + 1)],
                        start=(kt == 0),
                        stop=(kt == 1),
                    )
                ob = obp.tile([128, 512], f16, tag="ob", name="ob")
                nc.vector.tensor_copy(ob[:], acc[:])
                nc.sync.dma_start(
                    out_d[128 * tt : 128 * (tt + 1), 512 * nk : 512 * (nk + 1)],
                    ob[:],
                )

            def attn(c, hp, bg, bg_rate, bg_sched):
                # attention for chunk c, head pair hp (heads 2hp, 2hp+1);
                # pops one background thunk (next chunk's qkv / prev chunk's
                # proj) per j so the PE always has exp-independent work
                kth = kT_sb[hp]
                qth = qT_sb[hp]
                nj = 4 * c + 4
                ya = yps.tile([HD + 1, 512], f32, tag="ya", name="ya")
                yb = yps.tile([HD + 1, 512], f32, tag="yb", name="yb")
                for j in range(nj):
                    st = max(0, 128 * j - 512 * c)
                    diag = j >= 4 * c
                    s2 = s2p.tile([128, 1024], f32, tag="s2", name="s2")
                    pe_mask = diag and MASK_MODE == "pe"
                    for half in range(2):
                        c0 = 512 * half + st
                        nc.tensor.matmul(
                            s2[:, c0 : 512 * (half + 1)],
                            kth[HD * half : HD * half + HD,
                                128 * j : 128 * (j + 1)],
                            qth[HD * half : HD * half + HD,
                                512 * c + st : 512 * (c + 1)],
                            start=True,
                            stop=not pe_mask,
                            skip_group_check=pe_mask,
                        )
                        if pe_mask:
                            # causal mask: accumulate -30000 into the
                            # invalid triangle of the diagonal square
                            nc.tensor.matmul(
                                s2[:, c0 : c0 + 128],
                                ident_sb[:],
                                mask_sb[:],
                                start=False,
                                stop=True,
                                skip_group_check=True,
                            )
                    es = esp.tile([128, 1024], f16, tag="es", name="es")
                    if st and EXP_STRIDED:
                        # one strided activation covers both heads' trimmed
                        # ranges ([st:512] and [512+st:1024])
                        nc.scalar.activation(
                            out=es[:].rearrange("p (h q) -> p h q", h=2)[:, :, st:512],
                            in_=s2[:].rearrange("p (h q) -> p h q", h=2)[:, :, st:512],
                            func=AF.Exp, scale=float(SCALE),
                        )
                    elif st:
                        for half in range(2):
                            c0 = 512 * half + st
                            c1 = 512 * (half + 1)
                            nc.scalar.activation(
                                out=es[:, c0:c1], in_=s2[:, c0:c1],
                                func=AF.Exp, scale=float(SCALE),
                            )
                    else:
                        nc.scalar.activation(
                            out=es[:], in_=s2[:], func=AF.Exp,
                            scale=float(SCALE),
                        )
                    if diag and MASK_MODE == "dve":
                        for half in range(2):
                            c0 = 512 * half + st
                            nc.vector.tensor_mul(
                                es[:, c0 : c0 + 128],
                                es[:, c0 : c0 + 128],
                                mask_sb[:],
                            )
                    # pace background thunks between exp and PV: scores/exp
                    # stream un-gated, while anything PV consumes this
                    # iteration (v[j]) is emitted before the PV reads it
                    bg_sched[0] += bg_rate
                    while bg and bg_sched[0] >= 1.0:
                        bg_sched[0] -= 1.0
                        bg.pop(0)()
                    nc.tensor.matmul(
                        ya[:, st:512], v_sb[j][:, 2 * hp, :],
                        es[:, st:512],
                        start=(j == 0), stop=(j == nj - 1),
                    )
                    nc.tensor.matmul(
                        yb[:, st:512], v_sb[j][:, 2 * hp + 1, :],
                        es[:, 512 + st : 1024],
                        start=(j == 0), stop=(j == nj - 1),
                    )
                # normalize: yT[0:64] = y * broadcast(1/rowsum)
                for half, yy in ((0, ya), (1, yb)):
                    # copy rowsums out of PSUM first: the approx reciprocal's
                    # bitwise seed needs IEEE fp32 bits, not PSUM accumulator
                    # format
                    rsb = nrm.tile([1, 512], f32, tag="rsb", name="rsb")
                    nc.vector.tensor_copy(rsb[:], yy[HD : HD + 1, :])
                    r32 = nrm.tile([1, 512], f32, tag="r32", name="r32")
                    nc.vector.reciprocal_approx_fast(r32[:], rsb[:])
                    bs = nrm.tile([HD, 512], f32, tag="bs", name="bs")
                    nc.gpsimd.partition_broadcast(bs[:], r32[:])
                    nc.vector.tensor_mul(
                        yT_sb[hp][HD * half : HD * half + HD,
                                  512 * c : 512 * (c + 1)],
                        yy[0:HD, :],
                        bs[:],
                    )

            # ---- chunk 0 prelude: only what attn(0, hp0) needs; the two
            # qk chains run DMA-feed-gated, spin fillers keep the PE warm ----
            acc_q = mmp.tile([128, 512], f32, tag="mm", name="acc_q0")
            acc_k = mmp.tile([128, 512], f32, tag="mm", name="acc_k0")
            for et in range(ET):
                for jt, acc in ((0, acc_q), (2, acc_k)):
                    nc.tensor.matmul(
                        acc[:],
                        wqk_sb[et][:, 128 * jt : 128 * (jt + 1)],
                        xT_sb[et][:, 0:512],
                        start=(et == 0),
                        stop=(et == ET - 1),
                    )
                if et % 2:
                    spin_mm(1)
            nc.vector.tensor_scalar_add(qT_sb[0][:, 0:512], acc_q[:],
                                        bq_sb[:, 0:1])
            nc.vector.tensor_copy(kT_sb[0][:, 0:512], acc_k[:])

            # ---- main pipeline over chunks; all projections deferred to
            # the last chunk's window (largest exp load, least qkv left) ----
            for c in range(TC):
                bg = []
                if c == 0:
                    # v(0) first: PV(j) needs v[j] popped by iteration j; the
                    # scores/exp stream ahead of it un-gated
                    bg += [(lambda t0=tt: v_block(0, t0)) for tt in range(4)]
                    bg += [lambda: qk_block(0, 1), lambda: qk_block(0, 3)]
                if c + 1 < TC:
                    bg += [(lambda cc=c + 1, jj=jt: qk_block(cc, jj)) for jt in range(4)]
                    bg += [(lambda cc=c + 1, t0=tt: v_block(cc, t0))
                           for tt in range(4 * (c + 1), 4 * (c + 2))]
                else:
                    bg += [(lambda cc=pc, t0=tt, n0=nk: proj_block(cc, t0, n0))
                           for pc in range(3) for tt in range(4 * pc, 4 * pc + 4)
                           for nk in range(2)]
                nj_total = 2 * (4 * c + 4)
                bg_rate = len(bg) / nj_total
                if c == 0:
                    bg_rate = 2.0
                bg_sched = [0.0]
                attn(c, 0, bg, bg_rate, bg_sched)
                attn(c, 1, bg, bg_rate, bg_sched)
                while bg:
                    bg.pop(0)()
            # keep the PE clock warm across the last normalize gap
            spin2 = s2p.tile([128, 1024], f32, tag="s2", name="spin2")
            for _ in range(24):
                nc.tensor.matmul(spin2[:, 0:512], yT_sb[0][:, 1536:1664],
                                 yT_sb[0][:, 1536:2048], start=True, stop=True)
            # tail: last chunk's projections
            for cc in (3,):
                for tt in range(4 * cc, 4 * cc + 4):
                    for nk in range(2):
                        proj_block(cc, tt, nk)

    nc.finalize()
    return nc


def _host_constants():
    r = np.arange(128)[:, None]   # key within block
    c = np.arange(128)[None, :]   # query within block
    if MASK_MODE == "pe":
        # additive causal mask for the diagonal squares: -30000 where q < k
        mask = np.where(c < r, np.float16(MASK_NEG), np.float16(0.0))
    else:
        mask = (c >= r).astype(np.float16)
    ident = np.eye(128, dtype=np.float16)
    ones4 = np.ones((128, HPC, 1), dtype=np.float16)
    return mask, ident, ones4


def _make_in_maps(x, w_qkv, b_qkv):
    mask, ident, ones4 = _host_constants()
    in_maps = []
    for c in range(N_CORES):
        b, hg = divmod(c, HPC)
        j0 = DJ * hg
        xT = np.ascontiguousarray(np.asarray(x[b], dtype=np.float32).T)
        wq = w_qkv[:, j0 : j0 + DJ]
        wk = w_qkv[:, E + j0 : E + j0 + DJ]
        wqk = np.ascontiguousarray(
            np.concatenate([wq, wk], axis=1), dtype=np.float32
        )
        wv = np.ascontiguousarray(w_qkv[:, 2 * E + j0 : 2 * E + j0 + DJ],
                                  dtype=np.float32)
        bq = np.ascontiguousarray(
            np.asarray(b_qkv[j0 : j0 + DJ], dtype=np.float32).reshape(2, 128).T
        )
        in_maps.append(
            {
                "xT": xT.astype(np.float16),
                "wqk": wqk.astype(np.float16),
                "wv": wv.astype(np.float16),
                "wp": None,  # filled below (needs w_proj)
                "bq": bq,
                "mask": mask,
                "ident": ident,
                "ones4": ones4,
            }
        )
    return in_maps


def _get_exec():
    """Build the Bass module and a cached jitted SPMD callable (once)."""
    if "exec" in _STATE:
        return _STATE["exec"]

    import jax
    from concourse import bass2jax, mybir
    from jax.experimental.shard_map import shard_map
    from jax.sharding import Mesh, PartitionSpec

    nc = _build_nc()
    _STATE["nc"] = nc
    bass2jax.install_neuronx_cc_hook()

    partition_name = (
        nc.partition_id_tensor.name if nc.partition_id_tensor else None
    )
    in_names = []
    out_names = []
    out_avals = []
    zero_outs = []
    for alloc in nc.m.functions[0].allocations:
        if not isinstance(alloc, mybir.MemoryLocationSet):
            continue
        name = alloc.memorylocations[0].name
        if alloc.kind == "ExternalInput":
            if name != partition_name:
                in_names.append(name)
        elif alloc.kind == "ExternalOutput":
            shape = tuple(alloc.tensor_shape)
            dtype = mybir.dt.np(alloc.dtype)
            out_names.append(name)
            out_avals.append(jax.core.ShapedArray(shape, dtype))
            zero_outs.append(np.zeros(shape, dtype))
    n_params = len(in_names)
    all_names = in_names + out_names
    if partition_name is not None:
        all_names = all_names + [partition_name]

    def _make_body(k):
        def _body(*args):
            operands = list(args)
            if partition_name is not None:
                operands.append(bass2jax.partition_id_tensor())
            for _ in range(k):
                outs = bass2jax._bass_exec_p.bind(
                    *operands,
                    out_avals=tuple(out_avals),
                    in_names=tuple(all_names),
                    out_names=tuple(out_names),
                    lowering_input_output_aliases=(),
                    sim_require_finite=True,
                    sim_require_nnan=True,
                    nc=nc,
                )
            return tuple(outs)

        return _body

    devices = jax.devices()[:N_CORES]
    mesh = Mesh(np.asarray(devices), ("core",))
    n_all = n_params + len(out_names)

    def _make_sharded(k):
        return jax.jit(
            shard_map(
                _make_body(k),
                mesh=mesh,
                in_specs=(PartitionSpec("core"),) * n_all,
                out_specs=(PartitionSpec("core"),) * len(out_names),
                check_rep=False,
            ),
            keep_unused=True,
        )

    sharded = _make_sharded(1)

    state = {
        "make_sharded": _make_sharded,
        "jax": jax,
        "sharded": sharded,
        "in_names": in_names,
        "out_names": out_names,
        "out_avals": out_avals,
        "zeros_dev": [
            jax.device_put(
                np.zeros((N_CORES * z.shape[0], *z.shape[1:]), z.dtype)
            )
            for z in zero_outs
        ],
    }
    _STATE["exec"] = state
    return state


def _concat_inputs(in_maps):
    st = _get_exec()
    return [
        np.concatenate([np.asarray(in_maps[c][name]) for c in range(N_CORES)], axis=0)
        for name in st["in_names"]
    ]


def _run_device(concat_in):
    """concat_in: list of global (8*dim0, ...) arrays (np or jax). Returns
    list of per-core output dicts."""
    st = _get_exec()
    out_arrs = st["sharded"](*concat_in, *st["zeros_dev"])
    res = []
    for c in range(N_CORES):
        d = {}
        for i, name in enumerate(st["out_names"]):
            shp = st["out_avals"][i].shape
            d[name] = np.asarray(out_arrs[i]).reshape(N_CORES, *shp)[c]
        res.append(d)
    return res


def kernel(x, w_qkv, b_qkv, w_proj, b_proj):
    x = np.asarray(x, dtype=np.float32)
    w_qkv = np.asarray(w_qkv, dtype=np.float32)
    b_qkv = np.asarray(b_qkv, dtype=np.float32)
    w_proj = np.asarray(w_proj, dtype=np.float32)
    b_proj = np.asarray(b_proj, dtype=np.float32)

    in_maps = _make_in_maps(x, w_qkv, b_qkv)
    for c in range(N_CORES):
        _, hg = divmod(c, HPC)
        j0 = DJ * hg
        in_maps[c]["wp"] = np.ascontiguousarray(
            w_proj[j0 : j0 + DJ, :]
        ).astype(np.float16)

    results = _run_device(_concat_inputs(in_maps))

    out = np.zeros((B, T, E), dtype=np.float32)
    for c in range(N_CORES):
        out[c // HPC] += results[c]["out"].astype(np.float32)
    # fold b_v through the projection; b_k cancels inside softmax
    bias = b_proj + b_qkv[2 * E :] @ w_proj
    out += bias[None, None, :]
    return out
